# revision 31
# baseline (speedup 1.0000x reference)
"""Point-Transformer attention block on 8 Trainium2 NeuronCores.

Shards the points axis N across 8 cores (all ops are pointwise in N).
Per core: channels on SBUF partitions, pixels (k-major) on the free dim.
All matmuls run in bf16; inputs are converted + laid out k-major on the
host so every DMA is contiguous.

v2 vs baseline:
  - 1024-col bf16 matmuls for xn / w23t (half the MM count)
  - point-phase ops batched over tile PAIRS (512 points per op)
  - b3 folded into the post-softmax relu bias (softmax sums to 1 over k)
  - reciprocal_approx_fast for the softmax denominator
  - final bias+identity fused into one scalar_tensor_tensor
  - emission order interleaves independent matmuls between dependent
    ones so the PE HAM clock-gate stays warm
"""

import numpy as np
import ml_dtypes

B, CIN, N, K = 4, 64, 16384, 16
MID, OUT, PT, SHARE = 64, 128, 8, 8
G = MID // SHARE          # 8 softmax groups
NCORES = 8
NS = N // NCORES          # points per core per batch (2048)
TP = 256                  # points per tile
TPK = TP * K              # pixels per tile (4096)
NT = NS // TP             # tiles per batch (8)
NTILES = B * NT           # 32
NPAIRS = NTILES // 2      # 16
SP = 2 * TP               # points per pair (512)
BF16 = ml_dtypes.bfloat16


def _build_consts(w0, b0, w1, b1, w2, b2, w3, b3, pw1, pw2, cw1, cw2, cb2,
                  wout, bout):
    f32, bf = np.float32, BF16
    c = {}
    w0t = w0.T.astype(np.float32)                                   # [64,128]
    c["w0t2"] = np.ascontiguousarray(
        np.concatenate([w0t, w0t], axis=0), bf)                     # [128,128]
    c["w23t"] = np.ascontiguousarray(
        np.concatenate([w3, -w2], axis=0).T, bf)                    # [128,128]
    pw2t2 = np.concatenate([pw2, pw2], axis=0).T                    # [64,128]
    c["pw2d"] = np.ascontiguousarray(
        np.concatenate([pw2t2, pw2t2], axis=0), bf)                 # [128,128]
    pw1t = pw1.T.astype(np.float32)                                 # [8,64]
    pw1q = np.zeros((OUT, OUT), np.float32)
    for q in range(4):
        for s in range(2):
            pw1q[32 * q + 8 * s:32 * q + 8 * s + PT,
                 s * MID:s * MID + MID] = pw1t
    c["pw1q"] = np.ascontiguousarray(pw1q, bf)                      # [128,128]
    cw1r = cw1.reshape(G, MID, K)
    cw1s = cw1r.sum(-1)                                             # [8,64]
    c["cat"] = np.ascontiguousarray((cw1s @ w1).T, bf)              # [128,8]
    gktp = np.zeros((OUT, (K // 2) * G), np.float32)
    for kp in range(K // 2):
        gktp[0:MID, kp * G:(kp + 1) * G] = cw1r[:, :, 2 * kp].T
        gktp[MID:OUT, kp * G:(kp + 1) * G] = cw1r[:, :, 2 * kp + 1].T
    c["gktp"] = np.ascontiguousarray(gktp, bf)                      # [128,64]
    c["hb"] = np.ascontiguousarray((cw1s @ (b1 - b2))[:, None], f32)  # [8,1]
    c["cw2t"] = np.ascontiguousarray(cw2.T, bf)                     # [8,128]
    c["cb2"] = np.ascontiguousarray(cb2[:, None], f32)              # [128,1]
    # Sb[64p+m, n] = sum_k e[(m%8)*16+k, n]: group-sum + broadcast in one MM
    bsumb = np.zeros((OUT, OUT), np.float32)
    for p in range(2):
        for m in range(MID):
            bsumb[(m % G) * K:((m % G) + 1) * K, p * MID + m] = 1.0
    c["bsumb"] = np.ascontiguousarray(bsumb, bf)                    # [128,128]
    bksel2 = np.zeros((OUT, (K // 2) * OUT), np.float32)
    for kp in range(K // 2):
        for j in range(2):
            for m in range(MID):
                bksel2[(m % G) * K + 2 * kp + j,
                       kp * OUT + j * MID + m] = 1.0
    c["bksel2"] = np.ascontiguousarray(bksel2, bf)                  # [128,1024]
    fold2 = np.zeros((OUT, MID), np.float32)
    for j in range(2):
        for m in range(MID):
            fold2[j * MID + m, m] = 1.0
    c["fold2"] = np.ascontiguousarray(fold2, bf)                    # [128,64]
    c["woutt"] = np.ascontiguousarray(wout.T, bf)                   # [64,128]
    c["b0"] = np.ascontiguousarray(b0[:, None], f32)                # [128,1]
    # b3 is applied after the softmax-weighted k-sum: sum_k w_k = 1 per
    # group, so sum_k w_k*(x3pre+b3) = sum_k w_k*x3pre + b3.
    c["b3c"] = np.ascontiguousarray(b3[:, None], f32)               # [64,1]
    c["bout"] = np.ascontiguousarray(bout[:, None], f32)            # [128,1]
    return c


CONST_SHAPES = dict(
    w0t2=[OUT, OUT], w23t=[OUT, OUT], pw2d=[OUT, OUT], pw1q=[OUT, OUT],
    cat=[OUT, G], gktp=[OUT, K * G // 2], hb=[G, 1], cw2t=[G, OUT],
    cb2=[OUT, 1], bsumb=[OUT, OUT],
    bksel2=[OUT, K * OUT // 2], fold2=[OUT, MID], woutt=[MID, OUT],
    b0=[OUT, 1], b3c=[MID, 1], bout=[OUT, 1],
)
CONST_F32 = {"hb", "cb2", "b0", "b3c", "bout"}


def _build_program():
    import concourse.bass as bass
    import concourse.tile as tile
    from concourse import mybir
    from contextlib import ExitStack

    f32 = mybir.dt.float32
    bf16 = mybir.dt.bfloat16
    AF = mybir.ActivationFunctionType
    ALU = mybir.AluOpType

    nc = bass.Bass()
    feats_d = nc.declare_dram_parameter("feats", [B, 2 * CIN, NS * K // 2],
                                        bf16, isOutput=False)
    ppfs_d = nc.declare_dram_parameter("ppfs", [B, OUT, NT * 512], bf16,
                                       isOutput=False)
    cdram = {k: nc.declare_dram_parameter(
                 k, v, f32 if k in CONST_F32 else bf16, isOutput=False)
             for k, v in CONST_SHAPES.items()}
    out_d = nc.declare_dram_parameter("out", [B, OUT, NS], bf16, isOutput=True)

    with tile.TileContext(nc) as tc, ExitStack() as ctx:
        consts = ctx.enter_context(tc.tile_pool(name="consts", bufs=1))
        ct = {k: consts.tile_from(v[:], name=k) for k, v in cdram.items()}

        io = ctx.enter_context(tc.tile_pool(name="io", bufs=3))
        sb_xn = ctx.enter_context(tc.tile_pool(name="sb_xn", bufs=3))
        sb_r = ctx.enter_context(tc.tile_pool(name="sb_r", bufs=3))
        sb_x3 = ctx.enter_context(tc.tile_pool(name="sb_x3", bufs=3))
        pair_p = ctx.enter_context(tc.tile_pool(name="pair_p", bufs=2))
        pt_pool = ctx.enter_context(tc.tile_pool(name="pt", bufs=4))
        ps_pix = ctx.enter_context(tc.tile_pool(name="ps_pix", bufs=3,
                                                space="PSUM"))
        ps_wf = ctx.enter_context(tc.tile_pool(name="ps_wf", bufs=2,
                                               space="PSUM"))

        ios = {}       # tile -> (ft, pf)
        st1 = {}       # tile -> dict(xn, r)
        pairs = {}     # pair -> dict(x3p, ddp, xid, sm, h, e, en, ...)

        def emit_io(t):
            """Prefetch a tile's inputs one iteration ahead of use."""
            b, tt = divmod(t, NT)
            ft = io.tile([OUT, TPK // 2], bf16, tag="ft")
            nc.sync.dma_start(ft[:], feats_d[b, :, tt * TPK // 2:
                                             (tt + 1) * TPK // 2])
            pf = io.tile([OUT, 512], bf16, tag="pf")
            nc.sync.dma_start(pf[:], ppfs_d[b, :, tt * 512:(tt + 1) * 512])
            ios[t] = (ft, pf)

        def pair_of(t):
            q = t // 2
            if q not in pairs:
                x3p = pair_p.tile([OUT, 2 * TPK // 2], bf16, tag="x3p")
                ddp = pair_p.tile([OUT, 2 * TPK // 2], bf16, tag="ddp")
                xid = pair_p.tile([OUT, SP], bf16, tag="xid")
                pairs[q] = dict(q=q, x3p=x3p, ddp=ddp, xid=xid)
            return pairs[q]

        def emit_s1(t):
            """Pixel phase A: r = relu(pw1@ppf), xn = relu(w0@feats+b0)."""
            ft, pf = ios.pop(t)
            pr = pair_of(t)
            pi = t % 2

            r = sb_r.tile([OUT, TPK // 2], bf16, tag="r")
            # r: 4-way row-tiled pw1 (concurrent in PE quadrant rows)
            for i in range(2):
                p = ps_pix.tile([OUT, 1024], f32, tag="pix")
                for j in range(2):
                    q = 2 * i + j
                    nc.tensor.matmul(out=p[:, j * 512:(j + 1) * 512],
                                     lhsT=ct["pw1q"][32 * q:32 * q + 16, :],
                                     rhs=pf[32 * q:32 * q + 16, :],
                                     start=True, stop=True,
                                     tile_position=(32 * q, 0))
                nc.scalar.activation(
                    r[:, i * 1024:(i + 1) * 1024], p[:], AF.Relu)

            # xn = relu(w0 @ feats + b0): h0/h64 MMs interleaved so the two
            # 64-row groups stream concurrently in the PE array
            xn = sb_xn.tile([OUT, TPK], bf16, tag="xn")
            for half in range(2):
                pA = ps_pix.tile([OUT, 1024], f32, tag="pix", name="pA")
                pB = ps_pix.tile([OUT, 1024], f32, tag="pix", name="pB")
                for j in range(2):
                    cs = slice(half * 1024 + j * 512,
                               half * 1024 + (j + 1) * 512)
                    js = slice(j * 512, (j + 1) * 512)
                    nc.tensor.matmul(out=pA[:, js], lhsT=ct["w0t2"][0:CIN, :],
                                     rhs=ft[0:CIN, cs], start=True, stop=True)
                    nc.tensor.matmul(out=pB[:, js], lhsT=ct["w0t2"][CIN:OUT, :],
                                     rhs=ft[CIN:OUT, cs], start=True, stop=True)
                nc.scalar.activation(xn[:, half * 1024:(half + 1) * 1024],
                                     pA[:], AF.Relu, bias=ct["b0"][:])
                nc.vector.tensor_scalar(
                    xn[:, 2048 + half * 1024:2048 + (half + 1) * 1024],
                    pB[:], ct["b0"][:], 0.0, op0=ALU.add, op1=ALU.max)
                if half == 0:
                    # stash the k=0 slice (identity path + cat rhs)
                    nc.vector.tensor_copy(pr["xid"][:, pi * TP:(pi + 1) * TP],
                                          xn[:, 0:TP])
            st1[t] = dict(xn=xn, r=r)

        def emit_s1b(t):
            """Pixel phase B: psum = [w3@xn + ptf | -w2@xn + ptf] -> x3, d."""
            st = st1.pop(t)
            xn, r = st["xn"], st["r"]
            x3dd = sb_x3.tile([OUT, TPK], bf16, tag="x3dd")
            for c in range(4):
                p = ps_pix.tile([OUT, 1024], f32, tag="pix")
                for j in range(2):
                    cs = slice(c * 1024 + j * 512, c * 1024 + (j + 1) * 512)
                    nc.tensor.matmul(out=p[:, j * 512:(j + 1) * 512],
                                     lhsT=ct["w23t"][:], rhs=xn[:, cs],
                                     start=True, stop=False)
                for s in range(2):
                    nc.tensor.matmul(
                        out=p[:, s * 512:(s + 1) * 512],
                        lhsT=ct["pw2d"][s * MID:(s + 1) * MID, :],
                        rhs=r[s * MID:(s + 1) * MID, c * 512:(c + 1) * 512],
                        start=False, stop=True)
                xo = x3dd[:, c * 1024:(c + 1) * 1024]
                if c % 2 == 0:
                    nc.scalar.activation(xo, p[:], AF.Identity)
                else:
                    nc.vector.tensor_copy(xo, p[:])
            st["x3dd"] = x3dd
            st1[t] = st

        def emit_repack(t):
            """Parity repack via SBUF->SBUF DMA into the pair tiles."""
            st = st1.pop(t)
            pr = pair_of(t)
            pi = t % 2
            x3dd = st["x3dd"]
            v = x3dd[:].rearrange("p (k2 two n) -> p two k2 n", two=2, n=TP)
            x3pv = pr["x3p"][:].rearrange("p (k2 pi n) -> p pi k2 n",
                                          pi=2, n=TP)
            ddpv = pr["ddp"][:].rearrange("p (k2 pi n) -> p pi k2 n",
                                          pi=2, n=TP)
            nc.sync.dma_start(x3pv[0:MID, pi], v[0:MID, 0])
            nc.sync.dma_start(x3pv[MID:OUT, pi], v[0:MID, 1])
            nc.sync.dma_start(ddpv[0:MID, pi], v[MID:OUT, 0])
            nc.sync.dma_start(ddpv[MID:OUT, pi], v[MID:OUT, 1])

        def emit_s2(q):
            """h = relu(sum_k cw1_k^T d_k + Ca x + hb) for a pair (512 pts)."""
            pr = pairs[q]
            hs = ps_wf.tile([OUT, 512], f32, tag="wf", name="hs")
            hps = hs[0:G, :]
            ddp = pr["ddp"]
            for kp in range(K // 2):
                nc.tensor.matmul(
                    out=hps, lhsT=ct["gktp"][:, kp * G:(kp + 1) * G],
                    rhs=ddp[:, kp * SP:(kp + 1) * SP],
                    start=(kp == 0), stop=False)
            nc.tensor.matmul(out=hps, lhsT=ct["cat"][:],
                             rhs=pr["xid"][:], start=False, stop=True)
            h = pt_pool.tile([G, SP], bf16, tag="h")
            nc.scalar.activation(h[:], hps, AF.Relu, bias=ct["hb"][:])
            pr["h"] = h

        def emit_cw2(q):
            # wl = cw2 @ h  (cb2 folded into the exp activation)
            pr = pairs[q]
            wl = ps_wf.tile([OUT, 512], f32, tag="wf", name="wl")
            nc.tensor.matmul(out=wl[:], lhsT=ct["cw2t"][:], rhs=pr["h"][:],
                             start=True, stop=True)
            pr["wl"] = wl

        def emit_exp(q):
            # e = exp(wl + cb2) — raw (un-normalized) softmax weights
            pr = pairs[q]
            e = pt_pool.tile([OUT, SP], bf16, tag="e")
            nc.scalar.activation(e[:], pr.pop("wl")[:], AF.Exp,
                                 bias=ct["cb2"][:])
            pr["e"] = e

        def emit_bsum(q):
            # Sb[64p+m, n] = sum_k e[(m%8)*16+k, n] (group-sum + broadcast)
            pr = pairs[q]
            sb = ps_wf.tile([OUT, 512], f32, tag="wf", name="sb")
            nc.tensor.matmul(out=sb[:], lhsT=ct["bsumb"][:], rhs=pr["e"][:],
                             start=True, stop=True)
            lnb = pt_pool.tile([OUT, SP], bf16, tag="lnb")
            nc.scalar.activation(lnb[:], sb[:], AF.Ln)
            pr["lnb"] = lnb

        def emit_rsu(q):
            # rsu = 1/Sb = exp(-ln Sb)
            pr = pairs[q]
            rsu = pt_pool.tile([OUT, SP], bf16, tag="rsu")
            nc.scalar.activation(rsu[:], pr.pop("lnb")[:], AF.Exp, scale=-1.0)
            pr["rsu"] = rsu

        def emit_wf(q, insts):
            """wf_kp = bksel2_kp @ e (raw); q2_kp = wf_kp * x3p_kp."""
            pr = pairs[q]
            if "q2" not in pr:
                pr["q2"] = pair_p.tile([OUT, 2 * TPK // 2], bf16, tag="q2",
                                       name="q2")
            q2, x3p, e = pr["q2"], pr["x3p"], pr["e"]
            for kp in insts:
                wfp = ps_wf.tile([OUT, 512], f32, tag="wf")
                nc.tensor.matmul(
                    out=wfp[:],
                    lhsT=ct["bksel2"][:, kp * OUT:(kp + 1) * OUT],
                    rhs=e[:], start=True, stop=True)
                cs = slice(kp * 512, (kp + 1) * 512)
                nc.vector.tensor_mul(q2[:, cs], wfp[:], x3p[:, cs])

        def emit_tree1(q):
            pr = pairs[q]
            q2 = pr["q2"]
            t1a = pt_pool.tile([OUT, 1024], bf16, tag="t1a")
            with nc.allow_low_precision("bf16 partial sums"):
                nc.gpsimd.tensor_add(t1a[:], q2[:, 0:1024], q2[:, 1024:2048])
            pr["t1a"] = t1a

        def emit_tree2(q):
            # finish the k-sum, then normalize: un = ueo * (1/S)
            pr = pairs[q]
            q2 = pr["q2"]
            t1b = pt_pool.tile([OUT, 1024], bf16, tag="t1b")
            t2 = pt_pool.tile([OUT, 1024], bf16, tag="t2")
            ueo = pt_pool.tile([OUT, 512], bf16, tag="ueo")
            with nc.allow_low_precision("bf16 partial sums"):
                nc.gpsimd.tensor_add(t1b[:], q2[:, 2048:3072], q2[:, 3072:4096])
                nc.gpsimd.tensor_add(t2[:], pr["t1a"][:], t1b[:])
                nc.gpsimd.tensor_add(ueo[:], t2[:, 0:512], t2[:, 512:1024])
            un = pt_pool.tile([OUT, SP], bf16, tag="un")
            nc.vector.tensor_mul(un[:], ueo[:], pr["rsu"][:])
            pr["un"] = un

        def emit_fold(q):
            # fold parity halves of un; o = relu(U + b3)
            pr = pairs[q]
            wop = ps_wf.tile([OUT, 512], f32, tag="wf")
            ups = wop[0:MID, :]
            nc.tensor.matmul(out=ups, lhsT=ct["fold2"][:], rhs=pr["un"][:],
                             start=True, stop=True)
            o = pt_pool.tile([MID, SP], bf16, tag="o")
            nc.scalar.activation(o[:], ups, AF.Relu, bias=ct["b3c"][:])
            pr["o"] = o
            pr["wop"] = wop

        def emit_out(q):
            # out = wout @ o + bout + x
            pr = pairs.pop(q)
            ops_ = pr["wop"][:, :]
            nc.tensor.matmul(out=ops_, lhsT=ct["woutt"][:], rhs=pr["o"][:],
                             start=True, stop=True)
            outt = pt_pool.tile([OUT, SP], bf16, tag="outt")
            nc.vector.scalar_tensor_tensor(
                out=outt[:], in0=ops_, scalar=ct["bout"][:], in1=pr["xid"][:],
                op0=ALU.add, op1=ALU.add)
            b, tt = divmod(2 * q, NT)
            nc.sync.dma_start(out_d[b, :, tt * TP:tt * TP + SP], outt[:])

        # pair q schedule: s2 @ 2q+2 | cw2/exp/bsum/rsu + wf[0:4] @ 2q+3 |
        #                  wf[4:8] + tree + un @ 2q+4 | fold/out @ 2q+5
        for it in range(NTILES):
            t = it

            def pq(off):
                q = (it - off) // 2
                return q if (it >= off and (it - off) % 2 == 0
                             and q < NPAIRS) else None

            q_s2, q_mid, q_wf2, q_out = pq(2), pq(3), pq(4), pq(5)

            if it == 0:
                emit_io(0)
            if it + 1 < NTILES:
                emit_io(it + 1)
            emit_s1(t)
            if q_s2 is not None:
                emit_s2(q_s2)
            if q_mid is not None:
                emit_cw2(q_mid)
                emit_exp(q_mid)
            if q_wf2 is not None:
                emit_wf(q_wf2, range(4, 8))
            if q_mid is not None:
                emit_bsum(q_mid)
            emit_s1b(t)
            emit_repack(t)
            if q_out is not None:
                emit_fold(q_out)
                emit_out(q_out)
            if q_mid is not None:
                emit_rsu(q_mid)
                emit_wf(q_mid, range(0, 4))
                emit_tree1(q_mid)
            if q_wf2 is not None:
                emit_tree2(q_wf2)

        # dense pipeline drain: pair 14 has wf2/out pending, pair 15 all
        emit_s2(NPAIRS - 1)
        emit_wf(NPAIRS - 2, range(4, 8))
        emit_cw2(NPAIRS - 1)
        emit_tree2(NPAIRS - 2)
        emit_exp(NPAIRS - 1)
        emit_fold(NPAIRS - 2)
        emit_out(NPAIRS - 2)
        emit_bsum(NPAIRS - 1)
        emit_rsu(NPAIRS - 1)
        emit_wf(NPAIRS - 1, range(0, 4))
        emit_tree1(NPAIRS - 1)
        emit_wf(NPAIRS - 1, range(4, 8))
        emit_tree2(NPAIRS - 1)
        emit_fold(NPAIRS - 1)
        emit_out(NPAIRS - 1)

    return nc


def _legalize_waits(nc):
    """This toolchain's walrus rejects >1 sync-wait per instruction; hoist
    extra waits onto same-engine event-semaphore instructions just before."""
    from concourse import mybir

    n_split = 0
    for fn in nc.m.functions:
        for bb in fn.blocks:
            insts = bb.instructions
            new_list = []
            for inst in insts:
                si = inst.sync_info
                if si is not None and si.on_wait is not None and len(si.on_wait) > 1:
                    waits = list(si.on_wait)
                    for j, w in enumerate(waits[:-1]):
                        ev = mybir.InstEventSemaphore(
                            name=f"{inst.name}-lw{j}", ins=[], outs=[])
                        ev.engine = inst.engine
                        ev.sync_info = mybir.SyncInfo(on_wait=[w], on_update=[])
                        new_list.append(ev)
                        n_split += 1
                    inst.sync_info = mybir.SyncInfo(
                        on_wait=[waits[-1]], on_update=list(si.on_update))
                new_list.append(inst)
            if len(new_list) != len(insts):
                insts[:] = new_list
    return n_split


def _pack_feats(x):
    # [B, CIN, NS, K] -> per-tile k-major cols, k-halves stacked on
    # partitions: out[b, h*64+c, t, k8*TP+n] = x[b, c, t*TP+n, h*8+k8]
    a = x.reshape(B, CIN, NT, TP, 2, K // 2).transpose(0, 4, 1, 2, 5, 3)
    return np.ascontiguousarray(a, BF16).reshape(B, 2 * CIN, NS * K // 2)


def _pack_ppfs(x):
    # [B, PT, NS, K] -> per-tile [128, 512] for 4-way row-tiled pw1:
    # partition 32q+8s+c holds pixel cols q*1024+s*512+j (j = k1*TP+n)
    a = x.reshape(B, PT, NT, TP, K).transpose(0, 1, 2, 4, 3)  # [B,8,NT,K,TP]
    out = np.zeros((B, OUT, NT, 512), BF16)
    for k in range(K):
        q, s, k1 = k // 4, (k // 2) % 2, k % 2
        out[:, 32 * q + 8 * s:32 * q + 8 * s + PT, :,
            k1 * TP:(k1 + 1) * TP] = a[:, :, :, k, :]
    return np.ascontiguousarray(out).reshape(B, OUT, NT * 512)


LAST_RESULTS = None


def kernel(sm_feats, sm_ppfs, w0, b0, w1, b1, w2, b2, w3, b3,
           pw1, pw2, cw1, cw2, cb2, wout, bout):
    global LAST_RESULTS
    from concourse.bass_utils import run_bass_kernel_spmd

    consts = _build_consts(w0, b0, w1, b1, w2, b2, w3, b3, pw1, pw2,
                           cw1, cw2, cb2, wout, bout)
    nc = _build_program()
    _legalize_waits(nc)

    in_maps = []
    for i in range(NCORES):
        sl = slice(i * NS, (i + 1) * NS)
        m = dict(consts)
        m["feats"] = _pack_feats(np.ascontiguousarray(sm_feats[:, :, sl, :]))
        m["ppfs"] = _pack_ppfs(np.ascontiguousarray(sm_ppfs[:, :, sl, :]))
        in_maps.append(m)

    res = run_bass_kernel_spmd(nc, in_maps, list(range(NCORES)))
    LAST_RESULTS = res
    shards = [res.results[i]["out"].astype(np.float32) for i in range(NCORES)]
    return np.concatenate(shards, axis=2)


# revision 32
# speedup vs baseline: 1.0080x; 1.0080x over previous
"""Point-Transformer attention block on 8 Trainium2 NeuronCores.

Shards the points axis N across 8 cores (all ops are pointwise in N).
Per core: channels on SBUF partitions, pixels (k-major) on the free dim.
All matmuls run in bf16; inputs are converted + laid out k-major on the
host so every DMA is contiguous.

v2 vs baseline:
  - 1024-col bf16 matmuls for xn / w23t (half the MM count)
  - point-phase ops batched over tile PAIRS (512 points per op)
  - b3 folded into the post-softmax relu bias (softmax sums to 1 over k)
  - reciprocal_approx_fast for the softmax denominator
  - final bias+identity fused into one scalar_tensor_tensor
  - emission order interleaves independent matmuls between dependent
    ones so the PE HAM clock-gate stays warm
"""

import numpy as np
import ml_dtypes

B, CIN, N, K = 4, 64, 16384, 16
MID, OUT, PT, SHARE = 64, 128, 8, 8
G = MID // SHARE          # 8 softmax groups
NCORES = 8
NS = N // NCORES          # points per core per batch (2048)
TP = 256                  # points per tile
TPK = TP * K              # pixels per tile (4096)
NT = NS // TP             # tiles per batch (8)
NTILES = B * NT           # 32
NPAIRS = NTILES // 2      # 16
SP = 2 * TP               # points per pair (512)
BF16 = ml_dtypes.bfloat16


def _build_consts(w0, b0, w1, b1, w2, b2, w3, b3, pw1, pw2, cw1, cw2, cb2,
                  wout, bout):
    f32, bf = np.float32, BF16
    c = {}
    w0t = w0.T.astype(np.float32)                                   # [64,128]
    c["w0t2"] = np.ascontiguousarray(
        np.concatenate([w0t, w0t], axis=0), bf)                     # [128,128]
    c["w23t"] = np.ascontiguousarray(
        np.concatenate([w3, -w2], axis=0).T, bf)                    # [128,128]
    pw2t2 = np.concatenate([pw2, pw2], axis=0).T                    # [64,128]
    c["pw2d"] = np.ascontiguousarray(
        np.concatenate([pw2t2, pw2t2], axis=0), bf)                 # [128,128]
    pw1t = pw1.T.astype(np.float32)                                 # [8,64]
    pw1q = np.zeros((OUT, OUT), np.float32)
    for q in range(4):
        for s in range(2):
            pw1q[32 * q + 8 * s:32 * q + 8 * s + PT,
                 s * MID:s * MID + MID] = pw1t
    c["pw1q"] = np.ascontiguousarray(pw1q, bf)                      # [128,128]
    cw1r = cw1.reshape(G, MID, K)
    cw1s = cw1r.sum(-1)                                             # [8,64]
    c["cat"] = np.ascontiguousarray((cw1s @ w1).T, bf)              # [128,8]
    gktp = np.zeros((OUT, (K // 2) * G), np.float32)
    for kp in range(K // 2):
        gktp[0:MID, kp * G:(kp + 1) * G] = cw1r[:, :, 2 * kp].T
        gktp[MID:OUT, kp * G:(kp + 1) * G] = cw1r[:, :, 2 * kp + 1].T
    c["gktp"] = np.ascontiguousarray(gktp, bf)                      # [128,64]
    c["hb"] = np.ascontiguousarray((cw1s @ (b1 - b2))[:, None], f32)  # [8,1]
    c["cw2t"] = np.ascontiguousarray(cw2.T, bf)                     # [8,128]
    c["cb2"] = np.ascontiguousarray(cb2[:, None], f32)              # [128,1]
    # Sb[64p+m, n] = sum_k e[(m%8)*16+k, n]: group-sum + broadcast in one MM
    bsumb = np.zeros((OUT, OUT), np.float32)
    for p in range(2):
        for m in range(MID):
            bsumb[(m % G) * K:((m % G) + 1) * K, p * MID + m] = 1.0
    c["bsumb"] = np.ascontiguousarray(bsumb, bf)                    # [128,128]
    bksel2 = np.zeros((OUT, (K // 2) * OUT), np.float32)
    for kp in range(K // 2):
        for j in range(2):
            for m in range(MID):
                bksel2[(m % G) * K + 2 * kp + j,
                       kp * OUT + j * MID + m] = 1.0
    c["bksel2"] = np.ascontiguousarray(bksel2, bf)                  # [128,1024]
    fold2 = np.zeros((OUT, MID), np.float32)
    for j in range(2):
        for m in range(MID):
            fold2[j * MID + m, m] = 1.0
    c["fold2"] = np.ascontiguousarray(fold2, bf)                    # [128,64]
    c["woutt"] = np.ascontiguousarray(wout.T, bf)                   # [64,128]
    c["b0"] = np.ascontiguousarray(b0[:, None], f32)                # [128,1]
    # b3 is applied after the softmax-weighted k-sum: sum_k w_k = 1 per
    # group, so sum_k w_k*(x3pre+b3) = sum_k w_k*x3pre + b3.
    c["b3c"] = np.ascontiguousarray(b3[:, None], f32)               # [64,1]
    c["bout"] = np.ascontiguousarray(bout[:, None], f32)            # [128,1]
    return c


CONST_SHAPES = dict(
    w0t2=[OUT, OUT], w23t=[OUT, OUT], pw2d=[OUT, OUT], pw1q=[OUT, OUT],
    cat=[OUT, G], gktp=[OUT, K * G // 2], hb=[G, 1], cw2t=[G, OUT],
    cb2=[OUT, 1], bsumb=[OUT, OUT],
    bksel2=[OUT, K * OUT // 2], fold2=[OUT, MID], woutt=[MID, OUT],
    b0=[OUT, 1], b3c=[MID, 1], bout=[OUT, 1],
)
CONST_F32 = {"hb", "cb2", "b0", "b3c", "bout"}


def _build_program():
    import concourse.bass as bass
    import concourse.tile as tile
    from concourse import mybir
    from contextlib import ExitStack

    f32 = mybir.dt.float32
    bf16 = mybir.dt.bfloat16
    AF = mybir.ActivationFunctionType
    ALU = mybir.AluOpType

    nc = bass.Bass()
    feats_d = nc.declare_dram_parameter("feats", [B, 2 * CIN, NS * K // 2],
                                        bf16, isOutput=False)
    ppfs_d = nc.declare_dram_parameter("ppfs", [B, OUT, NT * 512], bf16,
                                       isOutput=False)
    cdram = {k: nc.declare_dram_parameter(
                 k, v, f32 if k in CONST_F32 else bf16, isOutput=False)
             for k, v in CONST_SHAPES.items()}
    out_d = nc.declare_dram_parameter("out", [B, OUT, NS], bf16, isOutput=True)

    with tile.TileContext(nc) as tc, ExitStack() as ctx:
        consts = ctx.enter_context(tc.tile_pool(name="consts", bufs=1))
        ct = {k: consts.tile_from(v[:], name=k) for k, v in cdram.items()}

        io = ctx.enter_context(tc.tile_pool(name="io", bufs=3))
        sb_xn = ctx.enter_context(tc.tile_pool(name="sb_xn", bufs=2))
        sb_r = ctx.enter_context(tc.tile_pool(name="sb_r", bufs=2))
        sb_x3 = ctx.enter_context(tc.tile_pool(name="sb_x3", bufs=2))
        pair_p = ctx.enter_context(tc.tile_pool(name="pair_p", bufs=2))
        pt_pool = ctx.enter_context(tc.tile_pool(name="pt", bufs=4))
        ps_pix = ctx.enter_context(tc.tile_pool(name="ps_pix", bufs=3,
                                                space="PSUM"))
        ps_wf = ctx.enter_context(tc.tile_pool(name="ps_wf", bufs=2,
                                               space="PSUM"))

        ios = {}       # tile -> (ft, pf)
        st1 = {}       # tile -> dict(xn, r)
        pairs = {}     # pair -> dict(x3p, ddp, xid, sm, h, e, en, ...)

        def emit_io(t):
            """Prefetch a tile's inputs one iteration ahead of use."""
            b, tt = divmod(t, NT)
            ft = io.tile([OUT, TPK // 2], bf16, tag="ft")
            nc.sync.dma_start(ft[:], feats_d[b, :, tt * TPK // 2:
                                             (tt + 1) * TPK // 2])
            pf = io.tile([OUT, 512], bf16, tag="pf")
            nc.sync.dma_start(pf[:], ppfs_d[b, :, tt * 512:(tt + 1) * 512])
            ios[t] = (ft, pf)

        def pair_of(t):
            q = t // 2
            if q not in pairs:
                x3p = pair_p.tile([OUT, 2 * TPK // 2], bf16, tag="x3p")
                ddp = pair_p.tile([OUT, 2 * TPK // 2], bf16, tag="ddp")
                xid = pair_p.tile([OUT, SP], bf16, tag="xid")
                pairs[q] = dict(q=q, x3p=x3p, ddp=ddp, xid=xid)
            return pairs[q]

        def emit_s1(t):
            """Pixel phase A: r = relu(pw1@ppf), xn = relu(w0@feats+b0)."""
            ft, pf = ios.pop(t)
            pr = pair_of(t)
            pi = t % 2

            r = sb_r.tile([OUT, TPK // 2], bf16, tag="r")
            # r: 4-way row-tiled pw1 (concurrent in PE quadrant rows)
            for i in range(2):
                p = ps_pix.tile([OUT, 1024], f32, tag="pix")
                for j in range(2):
                    q = 2 * i + j
                    nc.tensor.matmul(out=p[:, j * 512:(j + 1) * 512],
                                     lhsT=ct["pw1q"][32 * q:32 * q + 16, :],
                                     rhs=pf[32 * q:32 * q + 16, :],
                                     start=True, stop=True,
                                     tile_position=(32 * q, 0))
                nc.scalar.activation(
                    r[:, i * 1024:(i + 1) * 1024], p[:], AF.Relu)

            # xn = relu(w0 @ feats + b0): h0/h64 MMs interleaved so the two
            # 64-row groups stream concurrently in the PE array
            xn = sb_xn.tile([OUT, TPK], bf16, tag="xn")
            for half in range(2):
                pA = ps_pix.tile([OUT, 1024], f32, tag="pix", name="pA")
                pB = ps_pix.tile([OUT, 1024], f32, tag="pix", name="pB")
                for j in range(2):
                    cs = slice(half * 1024 + j * 512,
                               half * 1024 + (j + 1) * 512)
                    js = slice(j * 512, (j + 1) * 512)
                    nc.tensor.matmul(out=pA[:, js], lhsT=ct["w0t2"][0:CIN, :],
                                     rhs=ft[0:CIN, cs], start=True, stop=True)
                    nc.tensor.matmul(out=pB[:, js], lhsT=ct["w0t2"][CIN:OUT, :],
                                     rhs=ft[CIN:OUT, cs], start=True, stop=True)
                nc.scalar.activation(xn[:, half * 1024:(half + 1) * 1024],
                                     pA[:], AF.Relu, bias=ct["b0"][:])
                nc.vector.tensor_scalar(
                    xn[:, 2048 + half * 1024:2048 + (half + 1) * 1024],
                    pB[:], ct["b0"][:], 0.0, op0=ALU.add, op1=ALU.max)
                if half == 0:
                    # stash the k=0 slice (identity path + cat rhs)
                    nc.vector.tensor_copy(pr["xid"][:, pi * TP:(pi + 1) * TP],
                                          xn[:, 0:TP])
            st1[t] = dict(xn=xn, r=r)

        def emit_s1b(t):
            """Pixel phase B: psum = [w3@xn + ptf | -w2@xn + ptf] -> x3, d."""
            st = st1.pop(t)
            xn, r = st["xn"], st["r"]
            x3dd = sb_x3.tile([OUT, TPK], bf16, tag="x3dd")
            for c in range(4):
                p = ps_pix.tile([OUT, 1024], f32, tag="pix")
                for j in range(2):
                    cs = slice(c * 1024 + j * 512, c * 1024 + (j + 1) * 512)
                    nc.tensor.matmul(out=p[:, j * 512:(j + 1) * 512],
                                     lhsT=ct["w23t"][:], rhs=xn[:, cs],
                                     start=True, stop=False)
                for s in range(2):
                    nc.tensor.matmul(
                        out=p[:, s * 512:(s + 1) * 512],
                        lhsT=ct["pw2d"][s * MID:(s + 1) * MID, :],
                        rhs=r[s * MID:(s + 1) * MID, c * 512:(c + 1) * 512],
                        start=False, stop=True)
                xo = x3dd[:, c * 1024:(c + 1) * 1024]
                if c % 2 == 0:
                    nc.scalar.activation(xo, p[:], AF.Identity)
                else:
                    nc.vector.tensor_copy(xo, p[:])
            st["x3dd"] = x3dd
            st1[t] = st

        def emit_repack(t):
            """Parity repack via SBUF->SBUF DMA into the pair tiles."""
            st = st1.pop(t)
            pr = pair_of(t)
            pi = t % 2
            x3dd = st["x3dd"]
            v = x3dd[:].rearrange("p (k2 two n) -> p two k2 n", two=2, n=TP)
            x3pv = pr["x3p"][:].rearrange("p (k2 pi n) -> p pi k2 n",
                                          pi=2, n=TP)
            ddpv = pr["ddp"][:].rearrange("p (k2 pi n) -> p pi k2 n",
                                          pi=2, n=TP)
            nc.sync.dma_start(x3pv[0:MID, pi], v[0:MID, 0])
            nc.sync.dma_start(x3pv[MID:OUT, pi], v[0:MID, 1])
            nc.sync.dma_start(ddpv[0:MID, pi], v[MID:OUT, 0])
            nc.sync.dma_start(ddpv[MID:OUT, pi], v[MID:OUT, 1])

        def emit_s2(q):
            """h = relu(sum_k cw1_k^T d_k + Ca x + hb) for a pair (512 pts)."""
            pr = pairs[q]
            hs = ps_wf.tile([OUT, 512], f32, tag="wf", name="hs")
            hps = hs[0:G, :]
            ddp = pr["ddp"]
            for kp in range(K // 2):
                nc.tensor.matmul(
                    out=hps, lhsT=ct["gktp"][:, kp * G:(kp + 1) * G],
                    rhs=ddp[:, kp * SP:(kp + 1) * SP],
                    start=(kp == 0), stop=False)
            nc.tensor.matmul(out=hps, lhsT=ct["cat"][:],
                             rhs=pr["xid"][:], start=False, stop=True)
            h = pt_pool.tile([G, SP], bf16, tag="h")
            nc.scalar.activation(h[:], hps, AF.Relu, bias=ct["hb"][:])
            pr["h"] = h

        def emit_cw2(q):
            # wl = cw2 @ h  (cb2 folded into the exp activation)
            pr = pairs[q]
            wl = ps_wf.tile([OUT, 512], f32, tag="wf", name="wl")
            nc.tensor.matmul(out=wl[:], lhsT=ct["cw2t"][:], rhs=pr["h"][:],
                             start=True, stop=True)
            pr["wl"] = wl

        def emit_exp(q):
            # e = exp(wl + cb2) — raw (un-normalized) softmax weights
            pr = pairs[q]
            e = pt_pool.tile([OUT, SP], bf16, tag="e")
            nc.scalar.activation(e[:], pr.pop("wl")[:], AF.Exp,
                                 bias=ct["cb2"][:])
            pr["e"] = e

        def emit_bsum(q):
            # Sb[64p+m, n] = sum_k e[(m%8)*16+k, n] (group-sum + broadcast)
            pr = pairs[q]
            sb = ps_wf.tile([OUT, 512], f32, tag="wf", name="sb")
            nc.tensor.matmul(out=sb[:], lhsT=ct["bsumb"][:], rhs=pr["e"][:],
                             start=True, stop=True)
            lnb = pt_pool.tile([OUT, SP], bf16, tag="lnb")
            nc.scalar.activation(lnb[:], sb[:], AF.Ln)
            pr["lnb"] = lnb

        def emit_rsu(q):
            # rsu = 1/Sb = exp(-ln Sb)
            pr = pairs[q]
            rsu = pt_pool.tile([OUT, SP], bf16, tag="rsu")
            nc.scalar.activation(rsu[:], pr.pop("lnb")[:], AF.Exp, scale=-1.0)
            pr["rsu"] = rsu

        def emit_wf(q, insts):
            """wf_kp = bksel2_kp @ e (raw); q2_kp = wf_kp * x3p_kp."""
            pr = pairs[q]
            if "q2" not in pr:
                pr["q2"] = pair_p.tile([OUT, 2 * TPK // 2], bf16, tag="q2",
                                       name="q2")
            q2, x3p, e = pr["q2"], pr["x3p"], pr["e"]
            for kp in insts:
                wfp = ps_wf.tile([OUT, 512], f32, tag="wf")
                nc.tensor.matmul(
                    out=wfp[:],
                    lhsT=ct["bksel2"][:, kp * OUT:(kp + 1) * OUT],
                    rhs=e[:], start=True, stop=True)
                cs = slice(kp * 512, (kp + 1) * 512)
                nc.vector.tensor_mul(q2[:, cs], wfp[:], x3p[:, cs])

        def emit_tree1(q):
            pr = pairs[q]
            q2 = pr["q2"]
            t1a = pt_pool.tile([OUT, 1024], bf16, tag="t1a")
            with nc.allow_low_precision("bf16 partial sums"):
                nc.gpsimd.tensor_add(t1a[:], q2[:, 0:1024], q2[:, 1024:2048])
            pr["t1a"] = t1a

        def emit_tree2(q):
            # finish the k-sum, then normalize: un = ueo * (1/S)
            pr = pairs[q]
            q2 = pr["q2"]
            t1b = pt_pool.tile([OUT, 1024], bf16, tag="t1b")
            t2 = pt_pool.tile([OUT, 1024], bf16, tag="t2")
            ueo = pt_pool.tile([OUT, 512], bf16, tag="ueo")
            with nc.allow_low_precision("bf16 partial sums"):
                nc.gpsimd.tensor_add(t1b[:], q2[:, 2048:3072], q2[:, 3072:4096])
                nc.gpsimd.tensor_add(t2[:], pr["t1a"][:], t1b[:])
                nc.gpsimd.tensor_add(ueo[:], t2[:, 0:512], t2[:, 512:1024])
            un = pt_pool.tile([OUT, SP], bf16, tag="un")
            nc.vector.tensor_mul(un[:], ueo[:], pr["rsu"][:])
            pr["un"] = un

        def emit_fold(q):
            # fold parity halves of un; o = relu(U + b3)
            pr = pairs[q]
            wop = ps_wf.tile([OUT, 512], f32, tag="wf")
            ups = wop[0:MID, :]
            nc.tensor.matmul(out=ups, lhsT=ct["fold2"][:], rhs=pr["un"][:],
                             start=True, stop=True)
            o = pt_pool.tile([MID, SP], bf16, tag="o")
            nc.scalar.activation(o[:], ups, AF.Relu, bias=ct["b3c"][:])
            pr["o"] = o
            pr["wop"] = wop

        def emit_out(q):
            # out = wout @ o + bout + x
            pr = pairs.pop(q)
            ops_ = pr["wop"][:, :]
            nc.tensor.matmul(out=ops_, lhsT=ct["woutt"][:], rhs=pr["o"][:],
                             start=True, stop=True)
            outt = pt_pool.tile([OUT, SP], bf16, tag="outt")
            nc.vector.scalar_tensor_tensor(
                out=outt[:], in0=ops_, scalar=ct["bout"][:], in1=pr["xid"][:],
                op0=ALU.add, op1=ALU.add)
            b, tt = divmod(2 * q, NT)
            nc.sync.dma_start(out_d[b, :, tt * TP:tt * TP + SP], outt[:])

        # pair q schedule: s2 @ 2q+2 | cw2/exp/bsum/rsu + wf[0:4] @ 2q+3 |
        #                  wf[4:8] + tree + un @ 2q+4 | fold/out @ 2q+5
        for it in range(NTILES):
            t = it

            def pq(off):
                q = (it - off) // 2
                return q if (it >= off and (it - off) % 2 == 0
                             and q < NPAIRS) else None

            q_s2, q_mid, q_wf2, q_out = pq(2), pq(3), pq(4), pq(5)

            if it == 0:
                emit_io(0)
            if it + 1 < NTILES:
                emit_io(it + 1)
            emit_s1(t)
            if q_s2 is not None:
                emit_s2(q_s2)
            if q_mid is not None:
                emit_cw2(q_mid)
                emit_exp(q_mid)
            if q_wf2 is not None:
                emit_wf(q_wf2, range(4, 8))
            if q_mid is not None:
                emit_bsum(q_mid)
            emit_s1b(t)
            emit_repack(t)
            if q_out is not None:
                emit_fold(q_out)
                emit_out(q_out)
            if q_mid is not None:
                emit_rsu(q_mid)
                emit_wf(q_mid, range(0, 4))
                emit_tree1(q_mid)
            if q_wf2 is not None:
                emit_tree2(q_wf2)

        # dense pipeline drain: pair 14 has wf2/out pending, pair 15 all
        emit_s2(NPAIRS - 1)
        emit_wf(NPAIRS - 2, range(4, 8))
        emit_cw2(NPAIRS - 1)
        emit_tree2(NPAIRS - 2)
        emit_exp(NPAIRS - 1)
        emit_fold(NPAIRS - 2)
        emit_out(NPAIRS - 2)
        emit_bsum(NPAIRS - 1)
        emit_rsu(NPAIRS - 1)
        emit_wf(NPAIRS - 1, range(0, 4))
        emit_tree1(NPAIRS - 1)
        emit_wf(NPAIRS - 1, range(4, 8))
        emit_tree2(NPAIRS - 1)
        emit_fold(NPAIRS - 1)
        emit_out(NPAIRS - 1)

    return nc


def _legalize_waits(nc):
    """This toolchain's walrus rejects >1 sync-wait per instruction; hoist
    extra waits onto same-engine event-semaphore instructions just before."""
    from concourse import mybir

    n_split = 0
    for fn in nc.m.functions:
        for bb in fn.blocks:
            insts = bb.instructions
            new_list = []
            for inst in insts:
                si = inst.sync_info
                if si is not None and si.on_wait is not None and len(si.on_wait) > 1:
                    waits = list(si.on_wait)
                    for j, w in enumerate(waits[:-1]):
                        ev = mybir.InstEventSemaphore(
                            name=f"{inst.name}-lw{j}", ins=[], outs=[])
                        ev.engine = inst.engine
                        ev.sync_info = mybir.SyncInfo(on_wait=[w], on_update=[])
                        new_list.append(ev)
                        n_split += 1
                    inst.sync_info = mybir.SyncInfo(
                        on_wait=[waits[-1]], on_update=list(si.on_update))
                new_list.append(inst)
            if len(new_list) != len(insts):
                insts[:] = new_list
    return n_split


def _pack_feats(x):
    # [B, CIN, NS, K] -> per-tile k-major cols, k-halves stacked on
    # partitions: out[b, h*64+c, t, k8*TP+n] = x[b, c, t*TP+n, h*8+k8]
    a = x.reshape(B, CIN, NT, TP, 2, K // 2).transpose(0, 4, 1, 2, 5, 3)
    return np.ascontiguousarray(a, BF16).reshape(B, 2 * CIN, NS * K // 2)


def _pack_ppfs(x):
    # [B, PT, NS, K] -> per-tile [128, 512] for 4-way row-tiled pw1:
    # partition 32q+8s+c holds pixel cols q*1024+s*512+j (j = k1*TP+n)
    a = x.reshape(B, PT, NT, TP, K).transpose(0, 1, 2, 4, 3)  # [B,8,NT,K,TP]
    out = np.zeros((B, OUT, NT, 512), BF16)
    for k in range(K):
        q, s, k1 = k // 4, (k // 2) % 2, k % 2
        out[:, 32 * q + 8 * s:32 * q + 8 * s + PT, :,
            k1 * TP:(k1 + 1) * TP] = a[:, :, :, k, :]
    return np.ascontiguousarray(out).reshape(B, OUT, NT * 512)


LAST_RESULTS = None


def kernel(sm_feats, sm_ppfs, w0, b0, w1, b1, w2, b2, w3, b3,
           pw1, pw2, cw1, cw2, cb2, wout, bout):
    global LAST_RESULTS
    from concourse.bass_utils import run_bass_kernel_spmd

    consts = _build_consts(w0, b0, w1, b1, w2, b2, w3, b3, pw1, pw2,
                           cw1, cw2, cb2, wout, bout)
    nc = _build_program()
    _legalize_waits(nc)

    in_maps = []
    for i in range(NCORES):
        sl = slice(i * NS, (i + 1) * NS)
        m = dict(consts)
        m["feats"] = _pack_feats(np.ascontiguousarray(sm_feats[:, :, sl, :]))
        m["ppfs"] = _pack_ppfs(np.ascontiguousarray(sm_ppfs[:, :, sl, :]))
        in_maps.append(m)

    res = run_bass_kernel_spmd(nc, in_maps, list(range(NCORES)))
    LAST_RESULTS = res
    shards = [res.results[i]["out"].astype(np.float32) for i in range(NCORES)]
    return np.concatenate(shards, axis=2)


# revision 33
# speedup vs baseline: 1.0123x; 1.0042x over previous
"""Point-Transformer attention block on 8 Trainium2 NeuronCores.

Shards the points axis N across 8 cores (all ops are pointwise in N).
Per core: channels on SBUF partitions, pixels (k-major) on the free dim.
All matmuls run in bf16; inputs are converted + laid out k-major on the
host so every DMA is contiguous.

Optimizations vs the original baseline (590us -> ~440us):
  - point-phase ops batched over tile PAIRS (512 points per op)
  - b3 folded into the post-softmax relu bias (softmax weights sum to 1
    over k), so the x3 psum evacuation is a pure cast
  - softmax normalization moved off the critical path: raw-e weights are
    applied to x3, the k-sum is normalized at the end via
    un = ueo * exp(-ln(Sb)); Sb comes from one fused group-sum+broadcast
    matmul (bsumb).  No slow DVE reciprocal anywhere.
  - final bias+identity fused into one scalar_tensor_tensor
  - xn h0/h64 row-group matmuls interleaved for PE concurrency
  - 3-deep pixel psum ring (6 banks) + 2-deep point psum ring (2 banks)
  - emission order: independent pixel matmuls first, point-phase matmuls
    spaced between dependent cross-engine hops; dense pipeline drain
"""

import numpy as np
import ml_dtypes

B, CIN, N, K = 4, 64, 16384, 16
MID, OUT, PT, SHARE = 64, 128, 8, 8
G = MID // SHARE          # 8 softmax groups
NCORES = 8
NS = N // NCORES          # points per core per batch (2048)
TP = 256                  # points per tile
TPK = TP * K              # pixels per tile (4096)
NT = NS // TP             # tiles per batch (8)
NTILES = B * NT           # 32
NPAIRS = NTILES // 2      # 16
SP = 2 * TP               # points per pair (512)
BF16 = ml_dtypes.bfloat16


def _build_consts(w0, b0, w1, b1, w2, b2, w3, b3, pw1, pw2, cw1, cw2, cb2,
                  wout, bout):
    f32, bf = np.float32, BF16
    c = {}
    w0t = w0.T.astype(np.float32)                                   # [64,128]
    c["w0t2"] = np.ascontiguousarray(
        np.concatenate([w0t, w0t], axis=0), bf)                     # [128,128]
    c["w23t"] = np.ascontiguousarray(
        np.concatenate([w3, -w2], axis=0).T, bf)                    # [128,128]
    pw2t2 = np.concatenate([pw2, pw2], axis=0).T                    # [64,128]
    c["pw2d"] = np.ascontiguousarray(
        np.concatenate([pw2t2, pw2t2], axis=0), bf)                 # [128,128]
    pw1t = pw1.T.astype(np.float32)                                 # [8,64]
    pw1q = np.zeros((OUT, OUT), np.float32)
    for q in range(4):
        for s in range(2):
            pw1q[32 * q + 8 * s:32 * q + 8 * s + PT,
                 s * MID:s * MID + MID] = pw1t
    c["pw1q"] = np.ascontiguousarray(pw1q, bf)                      # [128,128]
    cw1r = cw1.reshape(G, MID, K)
    cw1s = cw1r.sum(-1)                                             # [8,64]
    c["cat"] = np.ascontiguousarray((cw1s @ w1).T, bf)              # [128,8]
    gktp = np.zeros((OUT, (K // 2) * G), np.float32)
    for kp in range(K // 2):
        gktp[0:MID, kp * G:(kp + 1) * G] = cw1r[:, :, 2 * kp].T
        gktp[MID:OUT, kp * G:(kp + 1) * G] = cw1r[:, :, 2 * kp + 1].T
    c["gktp"] = np.ascontiguousarray(gktp, bf)                      # [128,64]
    c["hb"] = np.ascontiguousarray((cw1s @ (b1 - b2))[:, None], f32)  # [8,1]
    c["cw2t"] = np.ascontiguousarray(cw2.T, bf)                     # [8,128]
    c["cb2"] = np.ascontiguousarray(cb2[:, None], f32)              # [128,1]
    # Sb[64p+m, n] = sum_k e[(m%8)*16+k, n]: group-sum + broadcast in one MM
    bsumb = np.zeros((OUT, OUT), np.float32)
    for p in range(2):
        for m in range(MID):
            bsumb[(m % G) * K:((m % G) + 1) * K, p * MID + m] = 1.0
    c["bsumb"] = np.ascontiguousarray(bsumb, bf)                    # [128,128]
    bksel2 = np.zeros((OUT, (K // 2) * OUT), np.float32)
    for kp in range(K // 2):
        for j in range(2):
            for m in range(MID):
                bksel2[(m % G) * K + 2 * kp + j,
                       kp * OUT + j * MID + m] = 1.0
    c["bksel2"] = np.ascontiguousarray(bksel2, bf)                  # [128,1024]
    fold2 = np.zeros((OUT, MID), np.float32)
    for j in range(2):
        for m in range(MID):
            fold2[j * MID + m, m] = 1.0
    c["fold2"] = np.ascontiguousarray(fold2, bf)                    # [128,64]
    c["woutt"] = np.ascontiguousarray(wout.T, bf)                   # [64,128]
    c["b0"] = np.ascontiguousarray(b0[:, None], f32)                # [128,1]
    # b3 is applied after the softmax-weighted k-sum: sum_k w_k = 1 per
    # group, so sum_k w_k*(x3pre+b3) = sum_k w_k*x3pre + b3.
    c["b3c"] = np.ascontiguousarray(b3[:, None], f32)               # [64,1]
    c["bout"] = np.ascontiguousarray(bout[:, None], f32)            # [128,1]
    return c


CONST_SHAPES = dict(
    w0t2=[OUT, OUT], w23t=[OUT, OUT], pw2d=[OUT, OUT], pw1q=[OUT, OUT],
    cat=[OUT, G], gktp=[OUT, K * G // 2], hb=[G, 1], cw2t=[G, OUT],
    cb2=[OUT, 1], bsumb=[OUT, OUT],
    bksel2=[OUT, K * OUT // 2], fold2=[OUT, MID], woutt=[MID, OUT],
    b0=[OUT, 1], b3c=[MID, 1], bout=[OUT, 1],
)
CONST_F32 = {"hb", "cb2", "b0", "b3c", "bout"}


def _build_program():
    import concourse.bass as bass
    import concourse.tile as tile
    from concourse import mybir
    from contextlib import ExitStack

    f32 = mybir.dt.float32
    bf16 = mybir.dt.bfloat16
    AF = mybir.ActivationFunctionType
    ALU = mybir.AluOpType

    nc = bass.Bass()
    feats_d = nc.declare_dram_parameter("feats", [B, 2 * CIN, NS * K // 2],
                                        bf16, isOutput=False)
    ppfs_d = nc.declare_dram_parameter("ppfs", [B, OUT, NT * 512], bf16,
                                       isOutput=False)
    cdram = {k: nc.declare_dram_parameter(
                 k, v, f32 if k in CONST_F32 else bf16, isOutput=False)
             for k, v in CONST_SHAPES.items()}
    out_d = nc.declare_dram_parameter("out", [B, OUT, NS], bf16, isOutput=True)

    with tile.TileContext(nc) as tc, ExitStack() as ctx:
        consts = ctx.enter_context(tc.tile_pool(name="consts", bufs=1))
        ct = {k: consts.tile_from(v[:], name=k) for k, v in cdram.items()}

        io = ctx.enter_context(tc.tile_pool(name="io", bufs=3))
        sb_xn = ctx.enter_context(tc.tile_pool(name="sb_xn", bufs=2))
        sb_r = ctx.enter_context(tc.tile_pool(name="sb_r", bufs=2))
        sb_x3 = ctx.enter_context(tc.tile_pool(name="sb_x3", bufs=2))
        pair_p = ctx.enter_context(tc.tile_pool(name="pair_p", bufs=2))
        pt_pool = ctx.enter_context(tc.tile_pool(name="pt", bufs=4))
        ps_pix = ctx.enter_context(tc.tile_pool(name="ps_pix", bufs=3,
                                                space="PSUM"))
        ps_wf = ctx.enter_context(tc.tile_pool(name="ps_wf", bufs=2,
                                               space="PSUM"))

        ios = {}       # tile -> (ft, pf)
        st1 = {}       # tile -> dict(xn, r)
        pairs = {}     # pair -> dict(x3p, ddp, xid, sm, h, e, en, ...)

        def emit_io(t):
            """Prefetch a tile's inputs one iteration ahead of use."""
            b, tt = divmod(t, NT)
            ft = io.tile([OUT, TPK // 2], bf16, tag="ft")
            nc.sync.dma_start(ft[:], feats_d[b, :, tt * TPK // 2:
                                             (tt + 1) * TPK // 2])
            pf = io.tile([OUT, 512], bf16, tag="pf")
            nc.sync.dma_start(pf[:], ppfs_d[b, :, tt * 512:(tt + 1) * 512])
            ios[t] = (ft, pf)

        def pair_of(t):
            q = t // 2
            if q not in pairs:
                x3p = pair_p.tile([OUT, 2 * TPK // 2], bf16, tag="x3p")
                ddp = pair_p.tile([OUT, 2 * TPK // 2], bf16, tag="ddp")
                xid = pair_p.tile([OUT, SP], bf16, tag="xid")
                pairs[q] = dict(q=q, x3p=x3p, ddp=ddp, xid=xid)
            return pairs[q]

        def emit_s1(t):
            """Pixel phase A: r = relu(pw1@ppf), xn = relu(w0@feats+b0)."""
            ft, pf = ios.pop(t)
            pr = pair_of(t)
            pi = t % 2

            r = sb_r.tile([OUT, TPK // 2], bf16, tag="r")
            # r: 4-way row-tiled pw1 (concurrent in PE quadrant rows)
            for i in range(2):
                p = ps_pix.tile([OUT, 1024], f32, tag="pix")
                for j in range(2):
                    q = 2 * i + j
                    nc.tensor.matmul(out=p[:, j * 512:(j + 1) * 512],
                                     lhsT=ct["pw1q"][32 * q:32 * q + 16, :],
                                     rhs=pf[32 * q:32 * q + 16, :],
                                     start=True, stop=True,
                                     tile_position=(32 * q, 0))
                nc.scalar.activation(
                    r[:, i * 1024:(i + 1) * 1024], p[:], AF.Relu)

            # xn = relu(w0 @ feats + b0): h0/h64 MMs interleaved so the two
            # 64-row groups stream concurrently in the PE array
            xn = sb_xn.tile([OUT, TPK], bf16, tag="xn")
            for half in range(2):
                pA = ps_pix.tile([OUT, 1024], f32, tag="pix", name="pA")
                pB = ps_pix.tile([OUT, 1024], f32, tag="pix", name="pB")
                for j in range(2):
                    cs = slice(half * 1024 + j * 512,
                               half * 1024 + (j + 1) * 512)
                    js = slice(j * 512, (j + 1) * 512)
                    nc.tensor.matmul(out=pA[:, js], lhsT=ct["w0t2"][0:CIN, :],
                                     rhs=ft[0:CIN, cs], start=True, stop=True)
                    nc.tensor.matmul(out=pB[:, js], lhsT=ct["w0t2"][CIN:OUT, :],
                                     rhs=ft[CIN:OUT, cs], start=True, stop=True)
                nc.scalar.activation(xn[:, half * 1024:(half + 1) * 1024],
                                     pA[:], AF.Relu, bias=ct["b0"][:])
                nc.vector.tensor_scalar(
                    xn[:, 2048 + half * 1024:2048 + (half + 1) * 1024],
                    pB[:], ct["b0"][:], 0.0, op0=ALU.add, op1=ALU.max)
                if half == 0:
                    # stash the k=0 slice (identity path + cat rhs)
                    nc.vector.tensor_copy(pr["xid"][:, pi * TP:(pi + 1) * TP],
                                          xn[:, 0:TP])
            st1[t] = dict(xn=xn, r=r)

        def emit_s1b(t):
            """Pixel phase B: psum = [w3@xn + ptf | -w2@xn + ptf] -> x3, d."""
            st = st1.pop(t)
            xn, r = st["xn"], st["r"]
            x3dd = sb_x3.tile([OUT, TPK], bf16, tag="x3dd")
            for c in range(4):
                p = ps_pix.tile([OUT, 1024], f32, tag="pix")
                for j in range(2):
                    cs = slice(c * 1024 + j * 512, c * 1024 + (j + 1) * 512)
                    nc.tensor.matmul(out=p[:, j * 512:(j + 1) * 512],
                                     lhsT=ct["w23t"][:], rhs=xn[:, cs],
                                     start=True, stop=False)
                for s in range(2):
                    nc.tensor.matmul(
                        out=p[:, s * 512:(s + 1) * 512],
                        lhsT=ct["pw2d"][s * MID:(s + 1) * MID, :],
                        rhs=r[s * MID:(s + 1) * MID, c * 512:(c + 1) * 512],
                        start=False, stop=True)
                xo = x3dd[:, c * 1024:(c + 1) * 1024]
                if c % 2 == 0:
                    nc.scalar.activation(xo, p[:], AF.Identity)
                else:
                    nc.vector.tensor_copy(xo, p[:])
            st["x3dd"] = x3dd
            st1[t] = st

        def emit_repack(t):
            """Parity repack via SBUF->SBUF DMA into the pair tiles."""
            st = st1.pop(t)
            pr = pair_of(t)
            pi = t % 2
            x3dd = st["x3dd"]
            v = x3dd[:].rearrange("p (k2 two n) -> p two k2 n", two=2, n=TP)
            x3pv = pr["x3p"][:].rearrange("p (k2 pi n) -> p pi k2 n",
                                          pi=2, n=TP)
            ddpv = pr["ddp"][:].rearrange("p (k2 pi n) -> p pi k2 n",
                                          pi=2, n=TP)
            nc.sync.dma_start(x3pv[0:MID, pi], v[0:MID, 0])
            nc.sync.dma_start(x3pv[MID:OUT, pi], v[0:MID, 1])
            nc.sync.dma_start(ddpv[0:MID, pi], v[MID:OUT, 0])
            nc.sync.dma_start(ddpv[MID:OUT, pi], v[MID:OUT, 1])

        def emit_s2(q):
            """h = relu(sum_k cw1_k^T d_k + Ca x + hb) for a pair (512 pts)."""
            pr = pairs[q]
            hs = ps_wf.tile([OUT, 512], f32, tag="wf", name="hs")
            hps = hs[0:G, :]
            ddp = pr["ddp"]
            for kp in range(K // 2):
                nc.tensor.matmul(
                    out=hps, lhsT=ct["gktp"][:, kp * G:(kp + 1) * G],
                    rhs=ddp[:, kp * SP:(kp + 1) * SP],
                    start=(kp == 0), stop=False)
            nc.tensor.matmul(out=hps, lhsT=ct["cat"][:],
                             rhs=pr["xid"][:], start=False, stop=True)
            h = pt_pool.tile([G, SP], bf16, tag="h")
            nc.scalar.activation(h[:], hps, AF.Relu, bias=ct["hb"][:])
            pr["h"] = h

        def emit_cw2(q):
            # wl = cw2 @ h  (cb2 folded into the exp activation)
            pr = pairs[q]
            wl = ps_wf.tile([OUT, 512], f32, tag="wf", name="wl")
            nc.tensor.matmul(out=wl[:], lhsT=ct["cw2t"][:], rhs=pr["h"][:],
                             start=True, stop=True)
            pr["wl"] = wl

        def emit_exp(q):
            # e = exp(wl + cb2) — raw (un-normalized) softmax weights
            pr = pairs[q]
            e = pt_pool.tile([OUT, SP], bf16, tag="e")
            nc.scalar.activation(e[:], pr.pop("wl")[:], AF.Exp,
                                 bias=ct["cb2"][:])
            pr["e"] = e

        def emit_bsum(q):
            # Sb[64p+m, n] = sum_k e[(m%8)*16+k, n] (group-sum + broadcast)
            pr = pairs[q]
            sb = ps_wf.tile([OUT, 512], f32, tag="wf", name="sb")
            nc.tensor.matmul(out=sb[:], lhsT=ct["bsumb"][:], rhs=pr["e"][:],
                             start=True, stop=True)
            lnb = pt_pool.tile([OUT, SP], bf16, tag="lnb")
            nc.scalar.activation(lnb[:], sb[:], AF.Ln)
            pr["lnb"] = lnb

        def emit_rsu(q):
            # rsu = 1/Sb = exp(-ln Sb)
            pr = pairs[q]
            rsu = pt_pool.tile([OUT, SP], bf16, tag="rsu")
            nc.scalar.activation(rsu[:], pr.pop("lnb")[:], AF.Exp, scale=-1.0)
            pr["rsu"] = rsu

        def emit_wf(q, insts):
            """wf_kp = bksel2_kp @ e (raw); q2_kp = wf_kp * x3p_kp."""
            pr = pairs[q]
            if "q2" not in pr:
                pr["q2"] = pair_p.tile([OUT, 2 * TPK // 2], bf16, tag="q2",
                                       name="q2")
            q2, x3p, e = pr["q2"], pr["x3p"], pr["e"]
            for kp in insts:
                wfp = ps_wf.tile([OUT, 512], f32, tag="wf")
                nc.tensor.matmul(
                    out=wfp[:],
                    lhsT=ct["bksel2"][:, kp * OUT:(kp + 1) * OUT],
                    rhs=e[:], start=True, stop=True)
                cs = slice(kp * 512, (kp + 1) * 512)
                nc.vector.tensor_mul(q2[:, cs], wfp[:], x3p[:, cs])

        def emit_tree1(q):
            pr = pairs[q]
            q2 = pr["q2"]
            t1a = pt_pool.tile([OUT, 1024], bf16, tag="t1a")
            with nc.allow_low_precision("bf16 partial sums"):
                nc.gpsimd.tensor_add(t1a[:], q2[:, 0:1024], q2[:, 1024:2048])
            pr["t1a"] = t1a

        def emit_tree2(q):
            # finish the k-sum, then normalize: un = ueo * (1/S)
            pr = pairs[q]
            q2 = pr["q2"]
            t1b = pt_pool.tile([OUT, 1024], bf16, tag="t1b")
            t2 = pt_pool.tile([OUT, 1024], bf16, tag="t2")
            ueo = pt_pool.tile([OUT, 512], bf16, tag="ueo")
            with nc.allow_low_precision("bf16 partial sums"):
                nc.gpsimd.tensor_add(t1b[:], q2[:, 2048:3072], q2[:, 3072:4096])
                nc.gpsimd.tensor_add(t2[:], pr["t1a"][:], t1b[:])
                nc.gpsimd.tensor_add(ueo[:], t2[:, 0:512], t2[:, 512:1024])
            un = pt_pool.tile([OUT, SP], bf16, tag="un")
            nc.vector.tensor_mul(un[:], ueo[:], pr["rsu"][:])
            pr["un"] = un

        def emit_fold(q):
            # fold parity halves of un; o = relu(U + b3)
            pr = pairs[q]
            wop = ps_wf.tile([OUT, 512], f32, tag="wf")
            ups = wop[0:MID, :]
            nc.tensor.matmul(out=ups, lhsT=ct["fold2"][:], rhs=pr["un"][:],
                             start=True, stop=True)
            o = pt_pool.tile([MID, SP], bf16, tag="o")
            nc.scalar.activation(o[:], ups, AF.Relu, bias=ct["b3c"][:])
            pr["o"] = o
            pr["wop"] = wop

        def emit_out(q):
            # out = wout @ o + bout + x
            pr = pairs.pop(q)
            ops_ = pr["wop"][:, :]
            nc.tensor.matmul(out=ops_, lhsT=ct["woutt"][:], rhs=pr["o"][:],
                             start=True, stop=True)
            outt = pt_pool.tile([OUT, SP], bf16, tag="outt")
            nc.vector.scalar_tensor_tensor(
                out=outt[:], in0=ops_, scalar=ct["bout"][:], in1=pr["xid"][:],
                op0=ALU.add, op1=ALU.add)
            b, tt = divmod(2 * q, NT)
            nc.sync.dma_start(out_d[b, :, tt * TP:tt * TP + SP], outt[:])

        # pair q schedule: s2 @ 2q+2 | cw2/exp/bsum/rsu + wf[0:4] @ 2q+3 |
        #                  wf[4:8] + tree + un @ 2q+4 | fold/out @ 2q+5
        for it in range(NTILES):
            t = it

            def pq(off):
                q = (it - off) // 2
                return q if (it >= off and (it - off) % 2 == 0
                             and q < NPAIRS) else None

            q_s2, q_mid, q_wf2, q_out = pq(2), pq(3), pq(4), pq(5)

            if it == 0:
                emit_io(0)
            if it + 1 < NTILES:
                emit_io(it + 1)
            emit_s1(t)
            if q_s2 is not None:
                emit_s2(q_s2)
            if q_mid is not None:
                emit_cw2(q_mid)
                emit_exp(q_mid)
            if q_wf2 is not None:
                emit_wf(q_wf2, range(4, 8))
            if q_mid is not None:
                emit_bsum(q_mid)
            emit_s1b(t)
            emit_repack(t)
            if q_out is not None:
                emit_fold(q_out)
                emit_out(q_out)
            if q_mid is not None:
                emit_rsu(q_mid)
                emit_wf(q_mid, range(0, 4))
                emit_tree1(q_mid)
            if q_wf2 is not None:
                emit_tree2(q_wf2)

        # dense pipeline drain: pair 14 has wf2/out pending, pair 15 all
        emit_s2(NPAIRS - 1)
        emit_wf(NPAIRS - 2, range(4, 8))
        emit_cw2(NPAIRS - 1)
        emit_tree2(NPAIRS - 2)
        emit_exp(NPAIRS - 1)
        emit_fold(NPAIRS - 2)
        emit_out(NPAIRS - 2)
        emit_bsum(NPAIRS - 1)
        emit_rsu(NPAIRS - 1)
        emit_wf(NPAIRS - 1, range(0, 4))
        emit_tree1(NPAIRS - 1)
        emit_wf(NPAIRS - 1, range(4, 8))
        emit_tree2(NPAIRS - 1)
        emit_fold(NPAIRS - 1)
        emit_out(NPAIRS - 1)

    return nc


def _legalize_waits(nc):
    """This toolchain's walrus rejects >1 sync-wait per instruction; hoist
    extra waits onto same-engine event-semaphore instructions just before."""
    from concourse import mybir

    n_split = 0
    for fn in nc.m.functions:
        for bb in fn.blocks:
            insts = bb.instructions
            new_list = []
            for inst in insts:
                si = inst.sync_info
                if si is not None and si.on_wait is not None and len(si.on_wait) > 1:
                    waits = list(si.on_wait)
                    for j, w in enumerate(waits[:-1]):
                        ev = mybir.InstEventSemaphore(
                            name=f"{inst.name}-lw{j}", ins=[], outs=[])
                        ev.engine = inst.engine
                        ev.sync_info = mybir.SyncInfo(on_wait=[w], on_update=[])
                        new_list.append(ev)
                        n_split += 1
                    inst.sync_info = mybir.SyncInfo(
                        on_wait=[waits[-1]], on_update=list(si.on_update))
                new_list.append(inst)
            if len(new_list) != len(insts):
                insts[:] = new_list
    return n_split


def _pack_feats(x):
    # [B, CIN, NS, K] -> per-tile k-major cols, k-halves stacked on
    # partitions: out[b, h*64+c, t, k8*TP+n] = x[b, c, t*TP+n, h*8+k8]
    a = x.reshape(B, CIN, NT, TP, 2, K // 2).transpose(0, 4, 1, 2, 5, 3)
    return np.ascontiguousarray(a, BF16).reshape(B, 2 * CIN, NS * K // 2)


def _pack_ppfs(x):
    # [B, PT, NS, K] -> per-tile [128, 512] for 4-way row-tiled pw1:
    # partition 32q+8s+c holds pixel cols q*1024+s*512+j (j = k1*TP+n)
    a = x.reshape(B, PT, NT, TP, K).transpose(0, 1, 2, 4, 3)  # [B,8,NT,K,TP]
    out = np.zeros((B, OUT, NT, 512), BF16)
    for k in range(K):
        q, s, k1 = k // 4, (k // 2) % 2, k % 2
        out[:, 32 * q + 8 * s:32 * q + 8 * s + PT, :,
            k1 * TP:(k1 + 1) * TP] = a[:, :, :, k, :]
    return np.ascontiguousarray(out).reshape(B, OUT, NT * 512)


LAST_RESULTS = None


def kernel(sm_feats, sm_ppfs, w0, b0, w1, b1, w2, b2, w3, b3,
           pw1, pw2, cw1, cw2, cb2, wout, bout):
    global LAST_RESULTS
    from concourse.bass_utils import run_bass_kernel_spmd

    consts = _build_consts(w0, b0, w1, b1, w2, b2, w3, b3, pw1, pw2,
                           cw1, cw2, cb2, wout, bout)
    nc = _build_program()
    _legalize_waits(nc)

    in_maps = []
    for i in range(NCORES):
        sl = slice(i * NS, (i + 1) * NS)
        m = dict(consts)
        m["feats"] = _pack_feats(np.ascontiguousarray(sm_feats[:, :, sl, :]))
        m["ppfs"] = _pack_ppfs(np.ascontiguousarray(sm_ppfs[:, :, sl, :]))
        in_maps.append(m)

    res = run_bass_kernel_spmd(nc, in_maps, list(range(NCORES)))
    LAST_RESULTS = res
    shards = [res.results[i]["out"].astype(np.float32) for i in range(NCORES)]
    return np.concatenate(shards, axis=2)
